# revision 8
# baseline (speedup 1.0000x reference)
"""Trainium2 Bass kernel for nn_BuildK (27-neighborhood kNN softmax weights).

Strategy: shard the y dimension across 8 NeuronCores (spatial parallel, no
cross-core communication). Each core receives a halo-extended, x-rotated input
slab, computes per-voxel: the 9 intensity-nearest of its 27 periodic neighbors
(stable selection network on f32 |diff| keys with an fp16 index payload),
reconstructs the sorted neighbor values exactly via sign*distance, forms the
pairwise feature-row distances through 27 shifted dot planes in f32, and
applies a rowwise softmax. Output is gathered and reassembled on the host.
"""

import sys

sys.path.insert(0, "/opt/trn_rl_repo")

import numpy as np

H, M, N = 64, 128, 128
NCORES = 8
YS = M // NCORES          # 16 owned y rows per core
YE = YS + 2               # 18 = sort region (owned + 1 halo each side)
YI = YS + 4               # 20 = input slab y extent (halo 2)
ZE = H + 2                # 66 = z extent with periodic wrap rows
KN = 9
EPS = 1e-6

POOL_CE_EVERY = 1000000         # every n'th comparator's min/max goes to gpsimd
POOL_DOT_EVERY = 2        # every n'th dot multiply goes to gpsimd
NSLOT = 40                # wire slots for the selection network


# --------------------------------------------------------------------------
# Selection network: top-9-sorted of 27, built from three 9-sorters and two
# pruned odd-even merges.  Ops are liveness-annotated for outputs 1..8
# (output 0 is always the center voxel: d=0, w=c).
# --------------------------------------------------------------------------

_SORT9 = [(0, 3), (1, 7), (2, 5), (4, 8), (0, 7), (2, 4), (3, 8), (5, 6),
          (0, 2), (1, 3), (4, 5), (7, 8), (1, 4), (3, 6), (5, 7), (0, 1),
          (2, 4), (3, 5), (6, 8), (2, 3), (4, 5), (6, 7), (1, 2), (3, 4),
          (5, 6)]


def _oddeven_merge(lo, n, r, out):
    step = r * 2
    if step < n:
        _oddeven_merge(lo, n, step, out)
        _oddeven_merge(lo + r, n, step, out)
        for i in range(lo + r, lo + n - r, step):
            out.append((i, i + r))
    else:
        out.append((lo, lo + r))


def _merge_topk(lenA, lenB, k):
    ces = []
    _oddeven_merge(0, 32, 1, ces)
    inf = [False] * 32
    for w in range(lenA, 16):
        inf[w] = True
    for w in range(16 + lenB, 32):
        inf[w] = True
    label = list(range(32))
    kept = []
    for (i, j) in ces:
        if inf[i] and inf[j]:
            continue
        if inf[j] and not inf[i]:
            continue
        if inf[i] and not inf[j]:
            label[i], label[j] = label[j], label[i]
            inf[i], inf[j] = False, True
            continue
        kept.append((label[i], label[j]))
    needed = set(label[w] for w in range(k))
    keep = []
    for (i, j) in reversed(kept):
        if i in needed or j in needed:
            keep.append((i, j))
            needed.add(i)
            needed.add(j)
    keep.reverse()

    def rm(w):
        return w if w < 16 else w - 16 + lenA

    return [(rm(i), rm(j)) for (i, j) in keep], [rm(label[w]) for w in range(k)]


def build_network():
    net = []
    for g in range(3):
        net += [(i + 9 * g, j + 9 * g) for (i, j) in _SORT9]
    m1, ow1 = _merge_topk(9, 9, 9)
    net += m1
    m2, ow2 = _merge_topk(9, 9, 9)
    remap = {i: ow1[i] for i in range(9)}
    remap.update({9 + i: 18 + i for i in range(9)})
    net += [(remap[i], remap[j]) for (i, j) in m2]
    outw = [remap[w] for w in ow2]

    live = set(outw[1:])
    ops = []
    for (i, j) in reversed(net):
        ni, nj = i in live, j in live
        if not (ni or nj):
            continue
        ops.append((i, j, ni, nj))
        live.add(i)
        live.add(j)
    ops.reverse()
    return ops, outw


NET_OPS, NET_OUTW = build_network()

OFFS = [(oz, oy, ox) for oz in (-1, 0, 1) for oy in (-1, 0, 1)
        for ox in (-1, 0, 1)]            # reference enumeration; 13 = center


# --------------------------------------------------------------------------
# Bass graph
# --------------------------------------------------------------------------

def build_bass(ks_value: float, reps: int = 1):
    from concourse import bacc, mybir
    from concourse import tile
    from concourse.alu_op_type import AluOpType as op

    f32 = mybir.dt.float32
    f16 = mybir.dt.float16
    AF = mybir.ActivationFunctionType

    nc = bacc.Bacc("TRN2", target_bir_lowering=False, debug=False,
                   num_devices=NCORES)

    xin = nc.dram_tensor("xin", [128, 3, ZE, YI], f32, kind="ExternalInput").ap()
    outd = nc.dram_tensor("out", [128, H, YS, KN], f32,
                          kind="ExternalOutput").ap()

    ZC = 16                      # z chunk for both phases
    FS = ZC * YE                 # 288 free elems in sort phase

    dve = nc.vector
    act = nc.scalar
    gp = nc.gpsimd

    ce_ctr = [0]

    def minmax_engine():
        ce_ctr[0] += 1
        return gp if (ce_ctr[0] % POOL_CE_EVERY == 0) else dve

    with tile.TileContext(nc) as tc:
      for _rep in range(reps):
        with tc.tile_pool(name="pp", bufs=1) as pp:
            Wslab = pp.tile([128, KN, ZE, YE], f32, tag="Wslab")
            idx9 = pp.tile([128, KN, H, YE], f16, tag="idx9")

            # ---------------- sort phase ----------------
            with tc.tile_pool(name="xp", bufs=1) as xp, \
                 tc.tile_pool(name="sortp", bufs=1) as sp:
                X3 = xp.tile([128, 3, ZE, YI], f32, tag="X3")
                nc.sync.dma_start(out=X3[:], in_=xin[:])
                kbig = sp.tile([128, NSLOT, FS], f32, tag="kbig")
                ibig = sp.tile([128, NSLOT, FS], f16, tag="ibig")

                for zc in range(0, H, ZC):
                    cvw = X3[:, 1, 1 + zc:1 + zc + ZC, 1:1 + YE]

                    def vview(d):
                        oz, oy, ox = OFFS[d]
                        return X3[:, ox + 1,
                                  1 + zc + oz:1 + zc + oz + ZC,
                                  1 + oy:1 + oy + YE]

                    free_slots = list(range(NSLOT))
                    wire_slot = {}

                    def k_ap(s):
                        return kbig[:, s, :]

                    def i_ap(s):
                        return ibig[:, s, :]

                    with tc.tile_pool(name="chunkp", bufs=1) as cp:
                        stmp = [cp.tile([128, FS], f32, name=f"s{i}", tag=f"s{i}")
                                for i in range(2)]
                        for d in range(27):
                            s = free_slots.pop()
                            wire_slot[d] = s
                            if d == 13:
                                dve.memset(k_ap(s), 0.0)
                            else:
                                st = stmp[d % 2]
                                eng = gp if d % 2 else dve
                                eng.tensor_tensor(out=st[:], in0=vview(d),
                                                  in1=cvw, op=op.subtract)
                                dve.scalar_tensor_tensor(
                                    out=k_ap(s), in0=st[:], scalar=-1.0,
                                    in1=st[:], op0=op.mult, op1=op.max)
                            dve.memset(i_ap(s), float(d))

                        NMT = 12
                        mt = [cp.tile([128, FS], f16, name=f"m{i}", tag=f"m{i}")
                              for i in range(NMT)]
                        bt = [cp.tile([128, FS], f16, name=f"b{i}", tag=f"b{i}")
                              for i in range(2 * NMT)]

                        for n, (i, j, ni, nj) in enumerate(NET_OPS):
                            si, sj = wire_slot[i], wire_slot[j]
                            m = mt[n % NMT]
                            w1 = bt[(n % NMT) * 2]
                            t = bt[(n % NMT) * 2 + 1]
                            dve.tensor_tensor(out=m[:], in0=k_ap(si),
                                              in1=k_ap(sj), op=op.is_le)
                            new_i = free_slots.pop() if ni else None
                            new_j = free_slots.pop() if nj else None
                            eng = minmax_engine()
                            if ni:
                                eng.tensor_tensor(out=k_ap(new_i),
                                                  in0=k_ap(si), in1=k_ap(sj),
                                                  op=op.min)
                            if nj:
                                eng.tensor_tensor(out=k_ap(new_j),
                                                  in0=k_ap(si), in1=k_ap(sj),
                                                  op=op.max)
                            dve.tensor_tensor(out=w1[:], in0=i_ap(si),
                                              in1=i_ap(sj), op=op.subtract)
                            dve.tensor_tensor(out=t[:], in0=w1[:], in1=m[:],
                                              op=op.mult)
                            if ni:
                                dve.tensor_tensor(out=i_ap(new_i), in0=t[:],
                                                  in1=i_ap(sj), op=op.add)
                            if nj:
                                dve.tensor_tensor(out=i_ap(new_j),
                                                  in0=i_ap(si), in1=t[:],
                                                  op=op.subtract)
                            free_slots.append(si)
                            free_slots.append(sj)
                            if ni:
                                wire_slot[i] = new_i
                            else:
                                del wire_slot[i]
                            if nj:
                                wire_slot[j] = new_j
                            else:
                                del wire_slot[j]

                        for r in range(1, KN):
                            dve.tensor_copy(
                                out=idx9[:, r, zc:zc + ZC, :],
                                in_=i_ap(wire_slot[NET_OUTW[r]]))

                        accs = [cp.tile([128, FS], f16, name=f"acc{r}", tag=f"acc{r}")
                                for r in range(1, KN)]
                        for a in accs:
                            dve.memset(a[:], 0.0)
                        sgn = [cp.tile([128, FS], f16, name=f"sg{i}", tag=f"sg{i}")
                               for i in range(2)]
                        occ = cp.tile([128, FS], f16, tag="occ")
                        for d in range(27):
                            if d == 13:
                                continue
                            st = stmp[d % 2]
                            eng = gp if d % 2 else dve
                            eng.tensor_tensor(out=st[:], in0=vview(d),
                                              in1=cvw, op=op.subtract)
                            sg = sgn[d % 2]
                            act.activation(out=sg[:], in_=st[:], func=AF.Sign)
                            for r in range(1, KN):
                                ir = i_ap(wire_slot[NET_OUTW[r]])
                                dve.tensor_scalar(out=occ[:], in0=ir,
                                                  scalar1=float(d),
                                                  scalar2=None,
                                                  op0=op.is_equal)
                                dve.tensor_tensor(out=occ[:], in0=occ[:],
                                                  in1=sg[:], op=op.mult)
                                dve.tensor_tensor(out=accs[r - 1][:],
                                                  in0=accs[r - 1][:],
                                                  in1=occ[:], op=op.add)

                        act.activation(out=Wslab[:, 0, 1 + zc:1 + zc + ZC, :],
                                       in_=cvw, func=AF.Copy)
                        sgf = cp.tile([128, FS], f32, tag="sgf")
                        wt = cp.tile([128, FS], f32, tag="wt")
                        for r in range(1, KN):
                            act.activation(out=sgf[:], in_=accs[r - 1][:],
                                           func=AF.Copy)
                            kr = k_ap(wire_slot[NET_OUTW[r]])
                            dve.tensor_tensor(out=wt[:], in0=sgf[:], in1=kr,
                                              op=op.mult)
                            dve.tensor_tensor(
                                out=Wslab[:, r, 1 + zc:1 + zc + ZC, :],
                                in0=wt[:], in1=cvw, op=op.add)

            # ---------------- z wrap rows of Wslab ----------------
            nc.sync.dma_start(out=Wslab[:, :, 0:1, :],
                              in_=Wslab[:, :, H:H + 1, :])
            nc.sync.dma_start(out=Wslab[:, :, ZE - 1:ZE, :],
                              in_=Wslab[:, :, 1:2, :])

            # ---------------- sigma / scale planes ----------------
            with tc.tile_pool(name="spp", bufs=1) as spp:
                B3 = spp.tile([128, 3, ZE, YE], f32, tag="B3")
                scalem = spp.tile([128, H, YS], f32, tag="scalem")
                Cp = spp.tile([128, H, YS], f32, tag="Cp")

                with tc.tile_pool(name="sigt", bufs=1) as sg2:
                    S1 = sg2.tile([128, ZE, YE], f32, tag="S1")
                    S2 = sg2.tile([128, ZE, YE], f32, tag="S2")
                    sq = sg2.tile([128, ZE, YE], f32, tag="sq")
                    dve.tensor_tensor(out=S1[:], in0=Wslab[:, 0],
                                      in1=Wslab[:, 1], op=op.add)
                    for r in range(2, KN):
                        gp.tensor_tensor(out=S1[:], in0=S1[:],
                                         in1=Wslab[:, r], op=op.add)
                    act.activation(out=S2[:], in_=Wslab[:, 0], func=AF.Square)
                    for r in range(1, KN):
                        act.activation(out=sq[:], in_=Wslab[:, r],
                                       func=AF.Square)
                        gp.tensor_tensor(out=S2[:], in0=S2[:], in1=sq[:],
                                         op=op.add)
                    dve.scalar_tensor_tensor(out=B3[:, 1], in0=S1[:],
                                             scalar=-2.0 * EPS, in1=S2[:],
                                             op0=op.mult, op1=op.add)
                    nc.sync.dma_start(out=B3[:, 0][1:128], in_=B3[:, 1][0:127])
                    nc.sync.dma_start(out=B3[:, 0][0:1], in_=B3[:, 1][127:128])
                    nc.sync.dma_start(out=B3[:, 2][0:127], in_=B3[:, 1][1:128])
                    nc.sync.dma_start(out=B3[:, 2][127:128], in_=B3[:, 1][0:1])

                    S1o = S1[:, 1:1 + H, 1:1 + YS]
                    S2o = S2[:, 1:1 + H, 1:1 + YS]
                    sq1 = sg2.tile([128, H, YS], f32, tag="sq1")
                    tvar = sg2.tile([128, H, YS], f32, tag="tvar")
                    tmpv = sg2.tile([128, H, YS], f32, tag="tmpv")
                    rec = sg2.tile([128, H, YS], f32, tag="rec")
                    act.activation(out=sq1[:], in_=S1o, func=AF.Square)
                    dve.scalar_tensor_tensor(out=tvar[:], in0=sq1[:],
                                             scalar=-1.0 / 9.0, in1=S2o,
                                             op0=op.mult, op1=op.add)
                    dve.tensor_scalar(out=tmpv[:], in0=tvar[:], scalar1=0.0,
                                      scalar2=None, op0=op.is_equal)
                    dve.tensor_tensor(out=tmpv[:], in0=tmpv[:], in1=tvar[:],
                                      op=op.add)
                    dve.reciprocal(out=rec[:], in_=tmpv[:])
                    dve.tensor_scalar(out=rec[:], in0=rec[:],
                                      scalar1=-4.0 / (ks_value * ks_value),
                                      scalar2=None, op0=op.mult)
                    dve.tensor_scalar(out=tmpv[:], in0=tvar[:], scalar1=0.0,
                                      scalar2=None, op0=op.not_equal)
                    dve.tensor_tensor(out=scalem[:], in0=rec[:], in1=tmpv[:],
                                      op=op.mult)
                    dve.scalar_tensor_tensor(out=Cp[:], in0=S1o,
                                             scalar=2.0 * EPS, in1=S2o,
                                             op0=op.mult, op1=op.add)
                    dve.tensor_scalar(out=Cp[:], in0=Cp[:],
                                      scalar1=9.0 * EPS * EPS, scalar2=None,
                                      op0=op.add)

                # ---------------- dots + softmax phase ----------------
                with tc.tile_pool(name="dotp", bufs=1) as dp:
                    for zc in range(0, H, ZC):
                        wr0 = dp.tile([128, KN, ZC + 2, YE], f32, tag="wr0")
                        wr2 = dp.tile([128, KN, ZC + 2, YE], f32, tag="wr2")
                        src = Wslab[:, :, zc:zc + ZC + 2, :]
                        nc.sync.dma_start(out=wr0[1:128], in_=src[0:127])
                        nc.sync.dma_start(out=wr0[0:1], in_=src[127:128])
                        nc.sync.dma_start(out=wr2[0:127], in_=src[1:128])
                        nc.sync.dma_start(out=wr2[127:128], in_=src[0:1])

                        est = dp.tile([128, 27, ZC, YS], f16, tag="est")
                        prodT = dp.tile([128, ZC * YS, KN], f32, tag="prodT")
                        pview = prodT[:].rearrange("p (z y) i -> p i z y",
                                                   z=ZC, y=YS)
                        dred = dp.tile([128, ZC, YS], f32, tag="dred")
                        t1 = dp.tile([128, ZC, YS], f32, tag="t1")
                        t2 = dp.tile([128, ZC, YS], f32, tag="t2")
                        argf = dp.tile([128, ZC, YS], f32, tag="argf")
                        scv = scalem[:, zc:zc + ZC, :]
                        cpv = Cp[:, zc:zc + ZC, :]

                        wA = Wslab[:, :, 1 + zc:1 + zc + ZC, 1:1 + YS]
                        for d in range(27):
                            oz, oy, ox = OFFS[d]
                            if ox == 0:
                                wB = Wslab[:, :,
                                           1 + zc + oz:1 + zc + oz + ZC,
                                           1 + oy:1 + oy + YS]
                            else:
                                wrt = wr0 if ox == -1 else wr2
                                wB = wrt[:, :, 1 + oz:1 + oz + ZC,
                                         1 + oy:1 + oy + YS]
                            eng = gp if (d % POOL_DOT_EVERY == 1) else dve
                            eng.tensor_tensor(out=pview, in0=wA, in1=wB,
                                              op=op.mult)
                            dve.tensor_reduce(out=dred[:], in_=prodT[:],
                                              axis=mybir.AxisListType.X,
                                              op=op.add)
                            Bv = B3[:, ox + 1,
                                    1 + zc + oz:1 + zc + oz + ZC,
                                    1 + oy:1 + oy + YS]
                            dve.tensor_tensor(out=t1[:], in0=Bv, in1=cpv,
                                              op=op.add)
                            dve.scalar_tensor_tensor(out=t2[:], in0=dred[:],
                                                     scalar=-2.0, in1=t1[:],
                                                     op0=op.mult, op1=op.add)
                            dve.tensor_tensor(out=argf[:], in0=t2[:],
                                              in1=scv, op=op.mult)
                            act.activation(out=est[:, d], in_=argf[:],
                                           func=AF.Exp)

                        e9 = [dp.tile([128, ZC, YS], f16, name=f"e9_{r}", tag=f"e9_{r}")
                              for r in range(1, KN)]
                        occ2 = dp.tile([128, ZC, YS], f16, tag="occ2")
                        for r in range(1, KN):
                            dve.memset(e9[r - 1][:], 0.0)
                            idv = idx9[:, r, zc:zc + ZC, 1:1 + YS]
                            for d in range(27):
                                dve.tensor_scalar(out=occ2[:], in0=idv,
                                                  scalar1=float(d),
                                                  scalar2=None,
                                                  op0=op.is_equal)
                                dve.tensor_tensor(out=occ2[:], in0=occ2[:],
                                                  in1=est[:, d], op=op.mult)
                                dve.tensor_tensor(out=e9[r - 1][:],
                                                  in0=e9[r - 1][:],
                                                  in1=occ2[:], op=op.add)

                        ssum = dp.tile([128, ZC, YS], f16, tag="ssum")
                        dve.tensor_tensor(out=ssum[:], in0=est[:, 13],
                                          in1=e9[0][:], op=op.add)
                        for r in range(2, KN):
                            dve.tensor_tensor(out=ssum[:], in0=ssum[:],
                                              in1=e9[r - 1][:], op=op.add)
                        sf = dp.tile([128, ZC, YS], f32, tag="sf")
                        act.activation(out=sf[:], in_=ssum[:], func=AF.Copy)
                        recs = dp.tile([128, ZC, YS], f32, tag="recs")
                        dve.reciprocal(out=recs[:], in_=sf[:])
                        rec16 = dp.tile([128, ZC, YS], f16, tag="rec16")
                        act.activation(out=rec16[:], in_=recs[:], func=AF.Copy)

                        ob = dp.tile([128, ZC, YS, KN], f32, tag="ob")
                        dve.tensor_tensor(out=ob[:, :, :, 0], in0=est[:, 13],
                                          in1=rec16[:], op=op.mult)
                        for r in range(1, KN):
                            dve.tensor_tensor(out=ob[:, :, :, r],
                                              in0=e9[r - 1][:], in1=rec16[:],
                                              op=op.mult)
                        nc.sync.dma_start(out=outd[:, zc:zc + ZC], in_=ob[:])

    nc.compile()
    return nc


# --------------------------------------------------------------------------
# Host side
# --------------------------------------------------------------------------

_CACHED = {}


def _get_nc(ks_value):
    key = float(ks_value)
    if key not in _CACHED:
        _CACHED[key] = build_bass(key)
    return _CACHED[key]


def _shard_inputs(x):
    """x: [H, M, N] f32 -> list of per-core xin arrays [128, 3, ZE, YI]."""
    maps = []
    zext = np.arange(-1, H + 1) % H
    xs = np.arange(N)
    for c in range(NCORES):
        ys = (np.arange(YS * c - 2, YS * c + YS + 2)) % M
        slab = x[zext][:, ys, :]                       # [66, 20, 128]
        a = np.empty((128, 3, ZE, YI), dtype=np.float32)
        for r in range(3):
            xrot = (xs + r - 1) % N
            a[:, r] = slab[:, :, xrot].transpose(2, 0, 1)
        maps.append({"xin": np.ascontiguousarray(a)})
    return maps


def kernel(input, ksigma, k, w):
    from concourse.bass_utils import run_bass_kernel_spmd

    x = np.asarray(input, dtype=np.float32)
    assert x.shape == (H, M, N)
    ks = float(np.asarray(ksigma).reshape(-1)[0])
    assert int(k) == KN and int(w) == 3

    nc = _get_nc(ks)
    in_maps = _shard_inputs(x)
    res = run_bass_kernel_spmd(nc, in_maps, core_ids=list(range(NCORES)))
    full = np.empty((H, M, N, KN), dtype=np.float32)
    for c in range(NCORES):
        oc = res.results[c]["out"]          # [128, H, YS, KN]
        full[:, YS * c:YS * c + YS] = oc.transpose(1, 2, 0, 3)
    return full.reshape(H * M * N, KN)


# revision 16
# speedup vs baseline: 1.5542x; 1.5542x over previous
"""Trainium2 Bass kernel for nn_BuildK (27-neighborhood kNN softmax weights).

Strategy: shard the y dimension across 8 NeuronCores (spatial parallel, no
cross-core communication). Each core receives a halo-extended, x-rotated input
slab, computes per-voxel: the 9 intensity-nearest of its 27 periodic neighbors
(stable selection network on f32 |diff| keys with an fp16 index payload),
reconstructs the sorted neighbor values exactly via sign*distance, forms the
pairwise feature-row distances through 27 shifted dot planes in f32, and
applies a rowwise softmax. Output is gathered and reassembled on the host.
"""

import sys

sys.path.insert(0, "/opt/trn_rl_repo")

import numpy as np

H, M, N = 64, 128, 128
NCORES = 8
YS = M // NCORES          # 16 owned y rows per core
YE = YS + 2               # 18 = sort region (owned + 1 halo each side)
YI = YS + 4               # 20 = input slab y extent (halo 2)
ZE = H + 2                # 66 = z extent with periodic wrap rows
KN = 9
EPS = 1e-6

POOL_CE_EVERY = 1000000   # min/max stay on DVE (Pool lacks min/max opcodes)
POOL_RANKS = (3,)         # apply-chain ranks owned by gpsimd
ACT_OCC = False           # build occ masks on ScalarE (2-op trick) for Pool ranks
DOT_DVE_EVERY = 3         # 1-in-n dot multiplies on DVE, rest on gpsimd
BLEND_POOL_EVERY = 2      # 1-in-n comparator blend groups on gpsimd
POOL_DOT_EVERY = 3        # every n'th dot multiply goes to gpsimd
NSLOT = 36                # wire slots for the selection network


# --------------------------------------------------------------------------
# Selection network: top-9-sorted of 27, built from three 9-sorters and two
# pruned odd-even merges.  Ops are liveness-annotated for outputs 1..8
# (output 0 is always the center voxel: d=0, w=c).
# --------------------------------------------------------------------------

_SORT9 = [(0, 3), (1, 7), (2, 5), (4, 8), (0, 7), (2, 4), (3, 8), (5, 6),
          (0, 2), (1, 3), (4, 5), (7, 8), (1, 4), (3, 6), (5, 7), (0, 1),
          (2, 4), (3, 5), (6, 8), (2, 3), (4, 5), (6, 7), (1, 2), (3, 4),
          (5, 6)]


def _oddeven_merge(lo, n, r, out):
    step = r * 2
    if step < n:
        _oddeven_merge(lo, n, step, out)
        _oddeven_merge(lo + r, n, step, out)
        for i in range(lo + r, lo + n - r, step):
            out.append((i, i + r))
    else:
        out.append((lo, lo + r))


def _merge_topk(lenA, lenB, k):
    ces = []
    _oddeven_merge(0, 32, 1, ces)
    inf = [False] * 32
    for w in range(lenA, 16):
        inf[w] = True
    for w in range(16 + lenB, 32):
        inf[w] = True
    label = list(range(32))
    kept = []
    for (i, j) in ces:
        if inf[i] and inf[j]:
            continue
        if inf[j] and not inf[i]:
            continue
        if inf[i] and not inf[j]:
            label[i], label[j] = label[j], label[i]
            inf[i], inf[j] = False, True
            continue
        kept.append((label[i], label[j]))
    needed = set(label[w] for w in range(k))
    keep = []
    for (i, j) in reversed(kept):
        if i in needed or j in needed:
            keep.append((i, j))
            needed.add(i)
            needed.add(j)
    keep.reverse()

    def rm(w):
        return w if w < 16 else w - 16 + lenA

    return [(rm(i), rm(j)) for (i, j) in keep], [rm(label[w]) for w in range(k)]


def build_network():
    """Top-8-sorted of the 26 non-center candidates.  Wires 0..25 map to
    candidates CAND[w] (reference enumeration minus the center 13).
    Returns (ops, outw, cand): ops = [(i, j, need_i, need_j)]."""
    cand = [d for d in range(27) if d != 13]
    S8 = [(0, 1), (2, 3), (4, 5), (6, 7), (0, 2), (1, 3), (4, 6), (5, 7),
          (1, 2), (5, 6), (0, 4), (3, 7), (1, 5), (2, 6), (1, 4), (3, 6),
          (2, 4), (3, 5), (3, 4)]
    net = []
    net += [(i, j) for (i, j) in _SORT9]
    net += [(i + 9, j + 9) for (i, j) in _SORT9]
    net += [(i + 18, j + 18) for (i, j) in S8]
    m1, ow1 = _merge_topk(9, 9, 8)
    net += m1
    m2, ow2 = _merge_topk(8, 8, 8)
    remap = {i: ow1[i] for i in range(8)}
    remap.update({8 + i: 18 + i for i in range(8)})
    net += [(remap[i], remap[j]) for (i, j) in m2]
    outw = [remap[w] for w in ow2]

    live = set(outw)
    ops = []
    for (i, j) in reversed(net):
        ni, nj = i in live, j in live
        if not (ni or nj):
            continue
        ops.append((i, j, ni, nj))
        live.add(i)
        live.add(j)
    ops.reverse()
    return ops, outw, cand


NET_OPS, NET_OUTW, CAND = build_network()

OFFS = [(oz, oy, ox) for oz in (-1, 0, 1) for oy in (-1, 0, 1)
        for ox in (-1, 0, 1)]            # reference enumeration; 13 = center


# --------------------------------------------------------------------------
# Bass graph
# --------------------------------------------------------------------------

def build_bass(ks_value: float, reps: int = 1):
    from concourse import bacc, mybir
    from concourse import tile
    from concourse.alu_op_type import AluOpType as op

    f32 = mybir.dt.float32
    f16 = mybir.dt.float16
    AF = mybir.ActivationFunctionType

    nc = bacc.Bacc("TRN2", target_bir_lowering=False, debug=False,
                   num_devices=NCORES)

    xin = nc.dram_tensor("xin", [128, 3, ZE, YI], f32, kind="ExternalInput").ap()
    outd = nc.dram_tensor("out", [128, H, YS, KN], f32,
                          kind="ExternalOutput").ap()

    ZC = 16                      # z chunk for both phases
    FS = ZC * YE                 # 288 free elems in sort phase

    dve = nc.vector
    act = nc.scalar
    gp = nc.gpsimd

    ce_ctr = [0]

    def minmax_engine():
        ce_ctr[0] += 1
        return gp if (ce_ctr[0] % POOL_CE_EVERY == 0) else dve

    with tile.TileContext(nc) as tc:
      for _rep in range(reps):
        with tc.tile_pool(name="pp", bufs=1) as pp:
            Wslab = pp.tile([128, KN, ZE, YE], f32, tag="Wslab")
            idx9 = pp.tile([128, KN, H, YE], f16, tag="idx9")
            cbias = pp.tile([128, 28], f32, tag="cbias")
            for d in range(27):
                dve.memset(cbias[:, d:d + 1], -float(d))
            dve.memset(cbias[:, 27:28], 1.0)
            cm1 = pp.tile([128, 1], f32, tag="cm1")
            dve.memset(cm1[:], -1.0)

            # ---------------- sort phase ----------------
            with tc.tile_pool(name="xp", bufs=1) as xp, \
                 tc.tile_pool(name="sortp", bufs=1) as sp:
                X3 = xp.tile([128, 3, ZE, YI], f32, tag="X3")
                nc.sync.dma_start(out=X3[:], in_=xin[:])
                kbig = sp.tile([128, NSLOT, FS], f32, tag="kbig")
                ibig = sp.tile([128, NSLOT, FS], f16, tag="ibig")

                for zc in range(0, H, ZC):
                    cvw = X3[:, 1, 1 + zc:1 + zc + ZC, 1:1 + YE]

                    def vview(d):
                        oz, oy, ox = OFFS[d]
                        return X3[:, ox + 1,
                                  1 + zc + oz:1 + zc + oz + ZC,
                                  1 + oy:1 + oy + YE]

                    free_slots = list(range(NSLOT))
                    wire_slot = {}

                    def k_ap(s):
                        return kbig[:, s, :]

                    def i_ap(s):
                        return ibig[:, s, :]

                    with tc.tile_pool(name="chunkp", bufs=1) as cp:
                        stmp = [cp.tile([128, FS], f32, name=f"s{i}", tag=f"s{i}")
                                for i in range(4)]
                        for w, d in enumerate(CAND):
                            s = free_slots.pop()
                            wire_slot[w] = s
                            st = stmp[d % 4]
                            eng = gp if d % 2 else dve
                            eng.tensor_tensor(out=st[:], in0=vview(d),
                                              in1=cvw, op=op.subtract)
                            dve.scalar_tensor_tensor(
                                out=k_ap(s), in0=st[:], scalar=-1.0,
                                in1=st[:], op0=op.mult, op1=op.max)
                            dve.memset(i_ap(s), float(d))

                        NMT = 12
                        mt = [cp.tile([128, FS], f16, name=f"m{i}", tag=f"m{i}")
                              for i in range(NMT)]
                        bt = [cp.tile([128, FS], f16, name=f"b{i}", tag=f"b{i}")
                              for i in range(2 * NMT)]

                        for n, (i, j, ni, nj) in enumerate(NET_OPS):
                            si, sj = wire_slot[i], wire_slot[j]
                            m = mt[n % NMT]
                            w1 = bt[(n % NMT) * 2]
                            t = bt[(n % NMT) * 2 + 1]
                            dve.tensor_tensor(out=m[:], in0=k_ap(si),
                                              in1=k_ap(sj), op=op.is_le)
                            new_i = free_slots.pop() if ni else None
                            new_j = free_slots.pop() if nj else None
                            eng = minmax_engine()
                            if ni:
                                eng.tensor_tensor(out=k_ap(new_i),
                                                  in0=k_ap(si), in1=k_ap(sj),
                                                  op=op.min)
                            if nj:
                                eng.tensor_tensor(out=k_ap(new_j),
                                                  in0=k_ap(si), in1=k_ap(sj),
                                                  op=op.max)
                            beng = gp if n % BLEND_POOL_EVERY == 0 else dve
                            beng.tensor_tensor(out=w1[:], in0=i_ap(si),
                                               in1=i_ap(sj), op=op.subtract)
                            beng.tensor_tensor(out=t[:], in0=w1[:], in1=m[:],
                                               op=op.mult)
                            if ni:
                                beng.tensor_tensor(out=i_ap(new_i),
                                                   in0=t[:], in1=i_ap(sj),
                                                   op=op.add)
                            if nj:
                                beng.tensor_tensor(out=i_ap(new_j),
                                                   in0=i_ap(si), in1=t[:],
                                                   op=op.subtract)
                            free_slots.append(si)
                            free_slots.append(sj)
                            if ni:
                                wire_slot[i] = new_i
                            else:
                                del wire_slot[i]
                            if nj:
                                wire_slot[j] = new_j
                            else:
                                del wire_slot[j]

                        for r in range(1, KN):
                            dve.tensor_copy(
                                out=idx9[:, r, zc:zc + ZC, :],
                                in_=i_ap(wire_slot[NET_OUTW[r - 1]]))

                        accs = [cp.tile([128, FS], f16, name=f"acc{r}", tag=f"acc{r}")
                                for r in range(1, KN)]
                        accs2 = [cp.tile([128, FS], f16, name=f"ac2{r}", tag=f"ac2{r}")
                                 for r in range(1, KN)]
                        sgn = [cp.tile([128, FS], f16, name=f"sg{i}", tag=f"sg{i}")
                               for i in range(4)]
                        occs = [cp.tile([128, FS], f16,
                                        name=f"occ{i}", tag=f"occ{i}")
                                for i in range(6)]
                        for d in CAND:
                            st = stmp[d % 4]
                            eng = gp if d % 2 else dve
                            eng.tensor_tensor(out=st[:], in0=vview(d),
                                              in1=cvw, op=op.subtract)
                            sg = sgn[d % 4]
                            act.activation(out=sg[:], in_=st[:], func=AF.Sign)
                            di = CAND.index(d)
                            first = di < 2
                            for r in range(1, KN):
                                ir = i_ap(wire_slot[NET_OUTW[r - 1]])
                                ach = gp if r in POOL_RANKS else dve
                                accr = accs[r - 1] if di % 2 == 0 else accs2[r - 1]
                                oc = occs[(r + d) % 6]
                                if (r in POOL_RANKS) and ACT_OCC:
                                    u = occs[(r + d + 3) % 6]
                                    act.activation(out=u[:], in_=ir,
                                                   func=AF.Abs,
                                                   bias=cbias[:, d:d + 1])
                                    act.activation(out=oc[:], in_=u[:],
                                                   func=AF.Relu,
                                                   scale=cm1[:],
                                                   bias=cbias[:, 27:28])
                                else:
                                    dve.tensor_scalar(out=oc[:], in0=ir,
                                                      scalar1=float(d),
                                                      scalar2=None,
                                                      op0=op.is_equal)
                                if first:
                                    ach.tensor_tensor(out=accr[:],
                                                      in0=oc[:], in1=sg[:],
                                                      op=op.mult)
                                else:
                                    ach.tensor_tensor(out=oc[:], in0=oc[:],
                                                      in1=sg[:], op=op.mult)
                                    ach.tensor_tensor(out=accr[:],
                                                      in0=accr[:],
                                                      in1=oc[:], op=op.add)

                        act.activation(out=Wslab[:, 0, 1 + zc:1 + zc + ZC, :],
                                       in_=cvw, func=AF.Copy)
                        sgfs = [cp.tile([128, FS], f32,
                                        name=f"sgf{i}", tag=f"sgf{i}")
                                for i in range(2)]
                        wts = [cp.tile([128, FS], f32,
                                       name=f"wt{i}", tag=f"wt{i}")
                               for i in range(2)]
                        for r in range(1, KN):
                            ach = gp if r in POOL_RANKS else dve
                            ach.tensor_tensor(out=accs[r - 1][:],
                                              in0=accs[r - 1][:],
                                              in1=accs2[r - 1][:], op=op.add)
                        for r in range(1, KN):
                            sgf = sgfs[r % 2]
                            wt = wts[r % 2]
                            act.activation(out=sgf[:], in_=accs[r - 1][:],
                                           func=AF.Copy)
                            kr = k_ap(wire_slot[NET_OUTW[r - 1]])
                            dve.tensor_tensor(out=wt[:], in0=sgf[:], in1=kr,
                                              op=op.mult)
                            dve.tensor_tensor(
                                out=Wslab[:, r, 1 + zc:1 + zc + ZC, :],
                                in0=wt[:], in1=cvw, op=op.add)

            # ---------------- z wrap rows of Wslab ----------------
            nc.sync.dma_start(out=Wslab[:, :, 0:1, :],
                              in_=Wslab[:, :, H:H + 1, :])
            nc.sync.dma_start(out=Wslab[:, :, ZE - 1:ZE, :],
                              in_=Wslab[:, :, 1:2, :])

            # ---------------- sigma / scale planes ----------------
            with tc.tile_pool(name="spp", bufs=1) as spp:
                B3 = spp.tile([128, 3, ZE, YE], f32, tag="B3")
                scalem = spp.tile([128, H, YS], f32, tag="scalem")
                Cp = spp.tile([128, H, YS], f32, tag="Cp")

                with tc.tile_pool(name="sigt", bufs=1) as sg2:
                    S1 = sg2.tile([128, ZE, YE], f32, tag="S1")
                    S2 = sg2.tile([128, ZE, YE], f32, tag="S2")
                    sq = sg2.tile([128, ZE, YE], f32, tag="sq")
                    dve.tensor_tensor(out=S1[:], in0=Wslab[:, 0],
                                      in1=Wslab[:, 1], op=op.add)
                    for r in range(2, KN):
                        gp.tensor_tensor(out=S1[:], in0=S1[:],
                                         in1=Wslab[:, r], op=op.add)
                    act.activation(out=S2[:], in_=Wslab[:, 0], func=AF.Square)
                    for r in range(1, KN):
                        act.activation(out=sq[:], in_=Wslab[:, r],
                                       func=AF.Square)
                        gp.tensor_tensor(out=S2[:], in0=S2[:], in1=sq[:],
                                         op=op.add)
                    dve.scalar_tensor_tensor(out=B3[:, 1], in0=S1[:],
                                             scalar=-2.0 * EPS, in1=S2[:],
                                             op0=op.mult, op1=op.add)
                    nc.sync.dma_start(out=B3[:, 0][1:128], in_=B3[:, 1][0:127])
                    nc.sync.dma_start(out=B3[:, 0][0:1], in_=B3[:, 1][127:128])
                    nc.sync.dma_start(out=B3[:, 2][0:127], in_=B3[:, 1][1:128])
                    nc.sync.dma_start(out=B3[:, 2][127:128], in_=B3[:, 1][0:1])

                    S1o = S1[:, 1:1 + H, 1:1 + YS]
                    S2o = S2[:, 1:1 + H, 1:1 + YS]
                    sq1 = sg2.tile([128, H, YS], f32, tag="sq1")
                    tvar = sg2.tile([128, H, YS], f32, tag="tvar")
                    tmpv = sg2.tile([128, H, YS], f32, tag="tmpv")
                    rec = sg2.tile([128, H, YS], f32, tag="rec")
                    act.activation(out=sq1[:], in_=S1o, func=AF.Square)
                    dve.scalar_tensor_tensor(out=tvar[:], in0=sq1[:],
                                             scalar=-1.0 / 9.0, in1=S2o,
                                             op0=op.mult, op1=op.add)
                    dve.tensor_scalar(out=tmpv[:], in0=tvar[:], scalar1=0.0,
                                      scalar2=None, op0=op.is_equal)
                    dve.tensor_tensor(out=tmpv[:], in0=tmpv[:], in1=tvar[:],
                                      op=op.add)
                    dve.reciprocal(out=rec[:], in_=tmpv[:])
                    dve.tensor_scalar(out=rec[:], in0=rec[:],
                                      scalar1=-4.0 / (ks_value * ks_value),
                                      scalar2=None, op0=op.mult)
                    dve.tensor_scalar(out=tmpv[:], in0=tvar[:], scalar1=0.0,
                                      scalar2=None, op0=op.not_equal)
                    dve.tensor_tensor(out=scalem[:], in0=rec[:], in1=tmpv[:],
                                      op=op.mult)
                    dve.scalar_tensor_tensor(out=Cp[:], in0=S1o,
                                             scalar=2.0 * EPS, in1=S2o,
                                             op0=op.mult, op1=op.add)
                    dve.tensor_scalar(out=Cp[:], in0=Cp[:],
                                      scalar1=9.0 * EPS * EPS, scalar2=None,
                                      op0=op.add)

                # ---------------- dots + softmax phase ----------------
                with tc.tile_pool(name="dotp", bufs=1) as dp:
                    for zc in range(0, H, ZC):
                        wr0 = dp.tile([128, KN, ZC + 2, YE], f32, tag="wr0")
                        wr2 = dp.tile([128, KN, ZC + 2, YE], f32, tag="wr2")
                        src = Wslab[:, :, zc:zc + ZC + 2, :]
                        nc.sync.dma_start(out=wr0[1:128], in_=src[0:127])
                        nc.sync.dma_start(out=wr0[0:1], in_=src[127:128])
                        nc.sync.dma_start(out=wr2[0:127], in_=src[1:128])
                        nc.sync.dma_start(out=wr2[127:128], in_=src[0:1])

                        est = dp.tile([128, 27, ZC, YS], f16, tag="est")
                        NR = 3
                        prodTs = [dp.tile([128, ZC * YS, KN], f32,
                                          name=f"prodT{i}", tag=f"prodT{i}")
                                  for i in range(NR)]
                        dreds = [dp.tile([128, ZC, YS], f32,
                                         name=f"dred{i}", tag=f"dred{i}")
                                 for i in range(NR)]
                        t1s = [dp.tile([128, ZC, YS], f32,
                                       name=f"t1_{i}", tag=f"t1_{i}")
                               for i in range(NR)]
                        t2s = [dp.tile([128, ZC, YS], f32,
                                       name=f"t2_{i}", tag=f"t2_{i}")
                               for i in range(NR)]
                        scv = scalem[:, zc:zc + ZC, :]
                        cpv = Cp[:, zc:zc + ZC, :]

                        wA = Wslab[:, :, 1 + zc:1 + zc + ZC, 1:1 + YS]
                        for d in range(27):
                            oz, oy, ox = OFFS[d]
                            if ox == 0:
                                wB = Wslab[:, :,
                                           1 + zc + oz:1 + zc + oz + ZC,
                                           1 + oy:1 + oy + YS]
                            else:
                                wrt = wr0 if ox == -1 else wr2
                                wB = wrt[:, :, 1 + oz:1 + oz + ZC,
                                         1 + oy:1 + oy + YS]
                            prodT = prodTs[d % NR]
                            pview = prodT[:].rearrange(
                                "p (z y) i -> p i z y", z=ZC, y=YS)
                            dred = dreds[d % NR]
                            t1 = t1s[d % NR]
                            t2 = t2s[d % NR]
                            eng = dve if d % DOT_DVE_EVERY == 1 else gp
                            eng.tensor_tensor(out=pview, in0=wA, in1=wB,
                                              op=op.mult)
                            dve.tensor_reduce(out=dred[:], in_=prodT[:],
                                              axis=mybir.AxisListType.X,
                                              op=op.add)
                            Bv = B3[:, ox + 1,
                                    1 + zc + oz:1 + zc + oz + ZC,
                                    1 + oy:1 + oy + YS]
                            dve.tensor_tensor(out=t1[:], in0=Bv, in1=cpv,
                                              op=op.add)
                            dve.scalar_tensor_tensor(out=t2[:], in0=dred[:],
                                                     scalar=-2.0, in1=t1[:],
                                                     op0=op.mult, op1=op.add)
                            dve.tensor_tensor(out=t1[:], in0=t2[:],
                                              in1=scv, op=op.mult)
                            act.activation(out=est[:, d], in_=t1[:],
                                           func=AF.Exp)

                        e9 = [dp.tile([128, ZC, YS], f16, name=f"e9_{r}", tag=f"e9_{r}")
                              for r in range(1, KN)]
                        e9b = [dp.tile([128, ZC, YS], f16, name=f"e9b_{r}", tag=f"e9b_{r}")
                               for r in range(1, KN)]
                        occ2s = [dp.tile([128, ZC, YS], f16,
                                         name=f"occ2_{i}", tag=f"occ2_{i}")
                                 for i in range(6)]
                        for r in range(1, KN):
                            idv = idx9[:, r, zc:zc + ZC, 1:1 + YS]
                            ach = gp if r in POOL_RANKS else dve
                            nd = 0
                            for d in range(27):
                                if d == 13:
                                    continue
                                first = nd < 2
                                er = e9[r - 1] if nd % 2 == 0 else e9b[r - 1]
                                nd += 1
                                oc = occ2s[(r + d) % 6]
                                if (r in POOL_RANKS) and ACT_OCC:
                                    u = occ2s[(r + d + 3) % 6]
                                    act.activation(out=u[:], in_=idv,
                                                   func=AF.Abs,
                                                   bias=cbias[:, d:d + 1])
                                    act.activation(out=oc[:], in_=u[:],
                                                   func=AF.Relu,
                                                   scale=cm1[:],
                                                   bias=cbias[:, 27:28])
                                else:
                                    dve.tensor_scalar(out=oc[:], in0=idv,
                                                      scalar1=float(d),
                                                      scalar2=None,
                                                      op0=op.is_equal)
                                if first:
                                    ach.tensor_tensor(out=er[:],
                                                      in0=oc[:],
                                                      in1=est[:, d],
                                                      op=op.mult)
                                else:
                                    ach.tensor_tensor(out=oc[:], in0=oc[:],
                                                      in1=est[:, d],
                                                      op=op.mult)
                                    ach.tensor_tensor(out=er[:],
                                                      in0=er[:],
                                                      in1=oc[:], op=op.add)

                        for r in range(1, KN):
                            ach = gp if r in POOL_RANKS else dve
                            ach.tensor_tensor(out=e9[r - 1][:],
                                              in0=e9[r - 1][:],
                                              in1=e9b[r - 1][:], op=op.add)
                        ssum = dp.tile([128, ZC, YS], f16, tag="ssum")
                        dve.tensor_tensor(out=ssum[:], in0=est[:, 13],
                                          in1=e9[0][:], op=op.add)
                        for r in range(2, KN):
                            dve.tensor_tensor(out=ssum[:], in0=ssum[:],
                                              in1=e9[r - 1][:], op=op.add)
                        sf = dp.tile([128, ZC, YS], f32, tag="sf")
                        act.activation(out=sf[:], in_=ssum[:], func=AF.Copy)
                        recs = dp.tile([128, ZC, YS], f32, tag="recs")
                        dve.reciprocal(out=recs[:], in_=sf[:])
                        rec16 = dp.tile([128, ZC, YS], f16, tag="rec16")
                        act.activation(out=rec16[:], in_=recs[:], func=AF.Copy)

                        ob = dp.tile([128, ZC, YS, KN], f32, tag="ob")
                        dve.tensor_tensor(out=ob[:, :, :, 0], in0=est[:, 13],
                                          in1=rec16[:], op=op.mult)
                        for r in range(1, KN):
                            dve.tensor_tensor(out=ob[:, :, :, r],
                                              in0=e9[r - 1][:], in1=rec16[:],
                                              op=op.mult)
                        nc.sync.dma_start(out=outd[:, zc:zc + ZC], in_=ob[:])

    nc.compile()
    return nc


# --------------------------------------------------------------------------
# Host side
# --------------------------------------------------------------------------

_CACHED = {}


def _get_nc(ks_value):
    key = float(ks_value)
    if key not in _CACHED:
        _CACHED[key] = build_bass(key)
    return _CACHED[key]


def _shard_inputs(x):
    """x: [H, M, N] f32 -> list of per-core xin arrays [128, 3, ZE, YI]."""
    maps = []
    zext = np.arange(-1, H + 1) % H
    xs = np.arange(N)
    for c in range(NCORES):
        ys = (np.arange(YS * c - 2, YS * c + YS + 2)) % M
        slab = x[zext][:, ys, :]                       # [66, 20, 128]
        a = np.empty((128, 3, ZE, YI), dtype=np.float32)
        for r in range(3):
            xrot = (xs + r - 1) % N
            a[:, r] = slab[:, :, xrot].transpose(2, 0, 1)
        maps.append({"xin": np.ascontiguousarray(a)})
    return maps


def kernel(input, ksigma, k, w):
    from concourse.bass_utils import run_bass_kernel_spmd

    x = np.asarray(input, dtype=np.float32)
    assert x.shape == (H, M, N)
    ks = float(np.asarray(ksigma).reshape(-1)[0])
    assert int(k) == KN and int(w) == 3

    nc = _get_nc(ks)
    in_maps = _shard_inputs(x)
    res = run_bass_kernel_spmd(nc, in_maps, core_ids=list(range(NCORES)))
    full = np.empty((H, M, N, KN), dtype=np.float32)
    for c in range(NCORES):
        oc = res.results[c]["out"]          # [128, H, YS, KN]
        full[:, YS * c:YS * c + YS] = oc.transpose(1, 2, 0, 3)
    return full.reshape(H * M * N, KN)
